# revision 1
# baseline (speedup 1.0000x reference)
"""TRN2 Bass kernel for nn_PosOnlyModel: 2-layer LSTM encoder (15 steps) +
autoregressive 2-layer LSTM decoder (25 steps) with Linear head and
unit-sphere residual position updates.

Strategy: pure data parallel across 8 NeuronCores (batch 8192 -> 1024/core,
weights replicated). Per core, feature-major layout: hidden state h kept as
[128, ktile, B] fp16 (matmul operand dtype), cell state c fp32. Per step,
each of the 8 gate M-tiles (permuted order [i0 f0 g0 o0 i1 f1 g1 o1])
accumulates its K-passes in PSUM; ACT applies sigmoid/tanh with fused
per-partition bias; DVE/GPSIMD combine the cell update. The decoder position
chain runs batch-major [128, 8, 3] on DVE with a bit-trick Newton rsqrt (no
ACT table switches), and the new position is transposed back to [3, B] via
the PE for the next step's K=3 input pass.
"""
import sys
import numpy as np

sys.path.insert(0, '/opt/trn_rl_repo')

import concourse.bass as bass
import concourse.bacc as bacc
import concourse.mybir as mybir
from concourse.tile import TileContext

dt = mybir.dt
F32 = dt.float32
MM = dt.float16
U32 = dt.uint32
AF = mybir.ActivationFunctionType
ALU = mybir.AluOpType
AX = mybir.AxisListType

H = 256
T_ENC = 15
T_DEC = 25
N_CORES = 8
B_FULL = 8192
B = B_FULL // N_CORES
RSQRT_MAGIC = 0x5f3759df


def build_kernel(repeats=1):
    nc = bacc.Bacc("TRN2", target_bir_lowering=False)
    NK = 2
    NC = B // 512
    NB = B // 128

    enc_x = nc.declare_dram_parameter("enc_x", [3, T_ENC * B], MM, isOutput=False)
    pos0 = nc.declare_dram_parameter("pos0", [3, B], MM, isOutput=False)
    pos0bm = nc.declare_dram_parameter("pos0bm", [128, NB * 3], F32, isOutput=False)
    w_hh, w_ih = {}, {}
    for cell in ("e0", "e1", "d0", "d1"):
        w_hh[cell] = nc.declare_dram_parameter(f"whh_{cell}", [128, NK * 8 * 128], MM, isOutput=False)
        if cell in ("e1", "d1"):
            w_ih[cell] = nc.declare_dram_parameter(f"wih_{cell}", [128, NK * 8 * 128], MM, isOutput=False)
        else:
            w_ih[cell] = nc.declare_dram_parameter(f"wih_{cell}", [128, 8 * 128], MM, isOutput=False)
    bias_d = nc.declare_dram_parameter("bias", [128, 4 * 8], F32, isOutput=False)
    fcw_d = nc.declare_dram_parameter("fcw", [128, NK * 4], MM, isOutput=False)
    fcbbm_d = nc.declare_dram_parameter("fcbbm", [128, NB * 3], F32, isOutput=False)
    magic_d = nc.declare_dram_parameter("magic", [128, NB], U32, isOutput=False)
    ident_d = nc.declare_dram_parameter("ident", [128, 128], F32, isOutput=False)
    ys_d = nc.declare_dram_parameter("ys", [128, T_DEC * NB * 3], F32, isOutput=True)

    with TileContext(nc) as tc:
        with tc.tile_pool(name="wpool", bufs=1) as wp, \
             tc.tile_pool(name="state", bufs=1) as sp, \
             tc.tile_pool(name="acts", bufs=10) as ap, \
             tc.tile_pool(name="tmps", bufs=6) as tp, \
             tc.tile_pool(name="xp", bufs=3) as xp, \
             tc.tile_pool(name="smalls", bufs=2) as mp, \
             tc.tile_pool(name="psum", bufs=4, space="PSUM") as pp:

            dmaeng = [nc.sync, nc.gpsimd, nc.scalar]
            di = [0]

            def dma(out, in_):
                dmaeng[di[0] % len(dmaeng)].dma_start(out=out, in_=in_)
                di[0] += 1

            whh_t, wih_t = {}, {}
            bias_t = wp.tile([128, 4 * 8], F32, name="bias")
            dma(bias_t[:], bias_d[:])
            for cell in ("e0", "d0"):
                wih_t[cell] = wp.tile([128, 8 * 128], MM, name=f"wih_{cell}")
                dma(wih_t[cell][:], w_ih[cell][:])
            for cell in ("e1", "d1"):
                wih_t[cell] = wp.tile([128, NK, 8 * 128], MM, name=f"wih_{cell}")
                dma(wih_t[cell][:], w_ih[cell][:].rearrange("p (k m) -> p k m", k=NK))
            for cell in ("e0", "e1", "d0", "d1"):
                whh_t[cell] = wp.tile([128, NK, 8 * 128], MM, name=f"whh_{cell}")
                dma(whh_t[cell][:], w_hh[cell][:].rearrange("p (k m) -> p k m", k=NK))
            fcw_t = wp.tile([128, NK, 4], MM, name="fcw")
            dma(fcw_t[:], fcw_d[:].rearrange("p (k m) -> p k m", k=NK))
            fcbbm_t = wp.tile([128, NB * 3], F32, name="fcbbm")
            dma(fcbbm_t[:], fcbbm_d[:])
            magic_t = wp.tile([128, NB], U32, name="magic")
            dma(magic_t[:], magic_d[:])
            ident_t = wp.tile([128, 128], F32, name="ident")
            dma(ident_t[:], ident_d[:])
            pos0bm_t = wp.tile([128, NB * 3], F32, name="pos0bm")
            dma(pos0bm_t[:], pos0bm[:])

            h_t = [[sp.tile([128, NK, B], MM, name=f"h{l}_{par}") for par in range(2)] for l in range(2)]
            c_t = [sp.tile([128, NK, B], F32, name=f"c{l}") for l in range(2)]

            CELLS = {0: ("e0", "e1"), 1: ("d0", "d1")}

            def emit_cell(cell, layer, s, x_ap, h_rd, h_wr, c_rw, bias_col):
                first = s == 0
                for hb in range(2):
                    acts = []
                    for mg in range(4):  # i f g o
                        m = hb * 4 + mg
                        ps = pp.tile([128, B], F32, name="ps_gates", tag="ps_gates")
                        passes = []
                        for (w, rhs) in h_rd:
                            for k in range(NK):
                                passes.append((w[:, k, m * 128:(m + 1) * 128], rhs[:, k, :]))
                        if layer == 0:
                            passes.append((wih_t[cell][0:3, m * 128:(m + 1) * 128], x_ap[0:3, :]))
                        n_pass = len(passes)
                        for pi, (w_ap, rhs) in enumerate(passes):
                            for ci in range(NC):
                                cs = slice(ci * 512, (ci + 1) * 512)
                                nc.tensor.matmul(ps[:, cs], w_ap, rhs[:, cs],
                                                 start=(pi == 0), stop=(pi == n_pass - 1))
                        a = ap.tile([128, B], F32, name="act", tag="act")
                        func = AF.Tanh if mg == 2 else AF.Sigmoid
                        nc.scalar.activation(a[:], ps[:], func, bias=bias_t[:, bias_col + m: bias_col + m + 1])
                        acts.append(a)
                    a_i, a_f, a_g, a_o = acts
                    c_dst = c_rw[:, hb, :]
                    t1 = tp.tile([128, B], F32, name="t1", tag="tmp")
                    t2 = tp.tile([128, B], F32, name="t2", tag="tmp")
                    t3 = tp.tile([128, B], F32, name="t3", tag="tmp")
                    for ci in range(NC):
                        cs = slice(ci * 512, (ci + 1) * 512)
                        if first:
                            nc.vector.tensor_mul(c_dst[:, cs], a_i[:, cs], a_g[:, cs])
                        else:
                            nc.vector.tensor_mul(t1[:, cs], a_f[:, cs], c_dst[:, cs])
                            nc.gpsimd.tensor_mul(t2[:, cs], a_i[:, cs], a_g[:, cs])
                            nc.vector.tensor_add(c_dst[:, cs], t1[:, cs], t2[:, cs])
                        nc.scalar.activation(t3[:, cs], c_dst[:, cs], AF.Tanh)
                        nc.vector.tensor_mul(h_wr[:, hb, cs], a_o[:, cs], t3[:, cs])

            for rep in range(repeats):
                pos_mm = None
                pos_bm = None
                for s in range(T_ENC + T_DEC):
                    is_dec = s >= T_ENC
                    p, q = s % 2, 1 - s % 2
                    c0n, c1n = CELLS[1 if is_dec else 0]
                    if is_dec:
                        if pos_mm is None:
                            pos_mm = mp.tile([3, B], MM, name="posmm", tag="smr")
                            nc.sync.dma_start(out=pos_mm[:], in_=pos0[:])
                            pos_bm = mp.tile([128, NB * 3], F32, name="pfcb", tag="pfcb")
                            nc.vector.tensor_add(pos_bm[:], pos0bm_t[:], fcbbm_t[:])
                        x_ap = pos_mm[:]
                    else:
                        xt = xp.tile([3, B], MM, name="xst", tag="xst")
                        nc.sync.dma_start(out=xt[:], in_=enc_x[:, s * B:(s + 1) * B])
                        x_ap = xt[:]
                    h_rd0 = [] if s == 0 else [(whh_t[c0n], h_t[0][q])]
                    emit_cell(c0n, 0, s, x_ap, h_rd0, h_t[0][p], c_t[0],
                              bias_col=(16 if is_dec else 0))
                    h_rd1 = [(wih_t[c1n], h_t[0][p])] if s == 0 else [(whh_t[c1n], h_t[1][q]), (wih_t[c1n], h_t[0][p])]
                    emit_cell(c1n, 1, s, None, h_rd1, h_t[1][p], c_t[1],
                              bias_col=(24 if is_dec else 8))
                    if is_dec:
                        d = s - T_ENC
                        NB_ = NB
                        psd = pp.tile([128, NB_, 4], F32, name="ps_fc", tag="ps_gates")
                        for m in range(NB_):
                            for k in range(NK):
                                nc.tensor.matmul(psd[:, m, :],
                                                 h_t[1][p][:, k, m * 128:(m + 1) * 128],
                                                 fcw_t[:, k, :],
                                                 start=(k == 0), stop=(k == NK - 1))
                        npos = mp.tile([128, NB_ * 3], F32, name="npos", tag="npos")
                        nc.vector.tensor_add(npos[:].rearrange("p (m c) -> p m c", c=3),
                                             psd[:, :, 0:3], pos_bm[:].rearrange("p (m c) -> p m c", c=3))
                        sq = mp.tile([128, NB_ * 3], F32, name="sq", tag="sm")
                        nc.vector.tensor_mul(sq[:], npos[:], npos[:])
                        ss = mp.tile([128, NB_], F32, name="ss", tag="sm8")
                        nc.vector.tensor_reduce(ss[:], sq[:].rearrange("p (m c) -> p m c", c=3),
                                                axis=AX.X, op=ALU.add)
                        u1 = mp.tile([128, NB_], U32, name="u1", tag="sm8b")
                        nc.vector.tensor_scalar(out=u1[:], in0=ss[:].bitcast(U32), scalar1=1, scalar2=None,
                                                op0=ALU.logical_shift_right)
                        y = mp.tile([128, NB_], F32, name="y", tag="sm8c")
                        nc.vector.tensor_tensor(out=y[:].bitcast(U32), in0=magic_t[:], in1=u1[:], op=ALU.subtract)
                        for _ in range(2):
                            t = mp.tile([128, NB_], F32, name="nrt", tag="sm8b")
                            nc.vector.tensor_mul(t[:], y[:], y[:])
                            nc.vector.tensor_mul(t[:], t[:], ss[:])
                            nc.vector.tensor_scalar(out=t[:], in0=t[:], scalar1=-0.5, scalar2=1.5,
                                                    op0=ALU.mult, op1=ALU.add)
                            nc.vector.tensor_mul(y[:], y[:], t[:])
                        posn = mp.tile([128, NB_ * 3], F32, name="posn", tag="ysst")
                        yb = y[:].unsqueeze(2).broadcast_to([128, NB_, 3])
                        nc.vector.tensor_tensor(out=posn[:].rearrange("p (m c) -> p m c", c=3),
                                                in0=npos[:].rearrange("p (m c) -> p m c", c=3),
                                                in1=yb, op=ALU.mult)
                        nc.sync.dma_start(out=ys_d[:, d * NB_ * 3:(d + 1) * NB_ * 3], in_=posn[:])
                        pos_mm = mp.tile([3, B], MM, name="posmm", tag="smr")
                        ps_tr = pp.tile([3, B], F32, name="ps_tr", tag="ps_gates")
                        for m in range(NB_):
                            nc.tensor.transpose(ps_tr[:, m * 128:(m + 1) * 128],
                                                posn[:, m * 3:(m + 1) * 3], ident_t[:])
                        for ci in range(NC):
                            cs = slice(ci * 512, (ci + 1) * 512)
                            nc.scalar.copy(out=pos_mm[:, cs], in_=ps_tr[:, cs])
                        pos_nb = mp.tile([128, NB_ * 3], F32, name="pfcb", tag="pfcb")
                        nc.vector.tensor_add(pos_nb[:], posn[:], fcbbm_t[:])
                        pos_bm = pos_nb

    nc.finalize()
    return nc


def pack_inputs(inputs):
    NB = B // 128
    perm = np.concatenate([np.arange(g * 256 + hb * 128, g * 256 + hb * 128 + 128)
                           for hb in range(2) for g in range(4)])

    def pack_whh(w):
        wt = w[perm].T.astype(np.float16)
        K = wt.shape[0]
        return wt.reshape(K // 128, 128, 1024).transpose(1, 0, 2).reshape(128, -1).copy()

    shared = {}
    for cell, pre in (("e0", "enc_"), ("e1", "enc_"), ("d0", "dec_"), ("d1", "dec_")):
        idx = cell[1]
        shared[f"whh_{cell}"] = pack_whh(np.asarray(inputs[f"{pre}Whh{idx}"]))
        wih = np.asarray(inputs[f"{pre}Wih{idx}"])
        if wih.shape[1] == H:
            shared[f"wih_{cell}"] = pack_whh(wih)
        else:
            w3 = wih[perm].T.astype(np.float16)
            rep = np.zeros((128, 8 * 128), np.float16)
            for j in range(4):
                rep[32 * j:32 * j + 3] = w3
            shared[f"wih_{cell}"] = rep
    bias = np.zeros((128, 32), np.float32)
    for j, (cell, pre) in enumerate((("e0", "enc_"), ("e1", "enc_"), ("d0", "dec_"), ("d1", "dec_"))):
        idx = cell[1]
        b = (np.asarray(inputs[f"{pre}bih{idx}"]) + np.asarray(inputs[f"{pre}bhh{idx}"]))[perm]
        bias[:, j * 8:(j + 1) * 8] = b.reshape(8, 128).T.astype(np.float32)
    shared["bias"] = bias
    fcw = np.zeros((256, 4), np.float16)
    fcw[:, :3] = np.asarray(inputs["fc_W"]).T.astype(np.float16)
    shared["fcw"] = fcw.reshape(2, 128, 4).transpose(1, 0, 2).reshape(128, 8).copy()
    shared["fcbbm"] = np.tile(np.asarray(inputs["fc_b"]).astype(np.float32), (128, NB)).copy()
    shared["magic"] = np.full((128, NB), RSQRT_MAGIC, np.uint32)
    shared["ident"] = np.eye(128, dtype=np.float32)

    enc = np.asarray(inputs["encoder_position_inputs"], np.float32)
    dec = np.asarray(inputs["decoder_position_inputs"], np.float32)
    in_maps = []
    for c in range(N_CORES):
        sl = slice(c * B, (c + 1) * B)
        m = dict(shared)
        m["enc_x"] = enc[sl].transpose(2, 1, 0).reshape(3, T_ENC * B).astype(np.float16)
        p0 = dec[sl, 0, :]
        m["pos0"] = p0.T.astype(np.float16)
        m["pos0bm"] = p0.reshape(NB, 128, 3).transpose(1, 0, 2).reshape(128, NB * 3).astype(np.float32).copy()
        in_maps.append(m)
    return in_maps


def unpack_outputs(results):
    NB = B // 128
    outs = []
    for c in range(N_CORES):
        ys = results[c]["ys"].reshape(128, T_DEC, NB, 3)
        outs.append(ys.transpose(2, 0, 1, 3).reshape(B, T_DEC, 3))
    return np.concatenate(outs, axis=0)


class SpmdRunner:
    """Compile a finalized Bass module once; run it many times."""

    def __init__(self, nc, n_cores):
        import jax
        from jax.sharding import Mesh, PartitionSpec
        from jax.experimental.shard_map import shard_map
        from concourse.bass2jax import _bass_exec_p, install_neuronx_cc_hook, partition_id_tensor
        self.jax = jax
        self.PartitionSpec = PartitionSpec
        install_neuronx_cc_hook()
        self.nc = nc
        self.n_cores = n_cores
        partition_name = nc.partition_id_tensor.name if nc.partition_id_tensor else None
        in_names, out_names, out_avals = [], [], []
        for alloc in nc.m.functions[0].allocations:
            if not isinstance(alloc, mybir.MemoryLocationSet):
                continue
            name = alloc.memorylocations[0].name
            if alloc.kind == "ExternalInput":
                if name != partition_name:
                    in_names.append(name)
            elif alloc.kind == "ExternalOutput":
                out_names.append(name)
                out_avals.append(jax.core.ShapedArray(tuple(alloc.tensor_shape), mybir.dt.np(alloc.dtype)))
        self.in_names, self.out_names, self.out_avals = in_names, out_names, out_avals
        n_params = len(in_names)
        n_outs = len(out_avals)
        all_in_names = list(in_names) + list(out_names)
        if partition_name is not None:
            all_in_names.append(partition_name)

        def _body(*args):
            operands = list(args)
            if partition_name is not None:
                operands.append(partition_id_tensor())
            outs = _bass_exec_p.bind(
                *operands,
                out_avals=tuple(out_avals),
                in_names=tuple(all_in_names),
                out_names=tuple(out_names),
                lowering_input_output_aliases=(),
                sim_require_finite=True,
                sim_require_nnan=True,
                nc=nc,
            )
            return tuple(outs)

        devices = jax.devices()[:n_cores]
        self.mesh = Mesh(np.asarray(devices), ("core",))
        in_specs = (PartitionSpec("core"),) * (n_params + n_outs)
        out_specs = (PartitionSpec("core"),) * n_outs
        donate = tuple(range(n_params, n_params + n_outs))
        self.sharded = jax.jit(
            shard_map(_body, mesh=self.mesh, in_specs=in_specs, out_specs=out_specs, check_rep=False),
            donate_argnums=donate, keep_unused=True,
        )
        self.n_params, self.n_outs = n_params, n_outs

    def __call__(self, in_maps, n_timed=0):
        import time
        jax = self.jax
        from jax.sharding import NamedSharding
        per_core = [[np.asarray(m[name]) for name in self.in_names] for m in in_maps]
        concat_in = [np.concatenate([per_core[c][i] for c in range(self.n_cores)], axis=0)
                     for i in range(self.n_params)]
        sh = NamedSharding(self.mesh, self.PartitionSpec("core"))
        concat_in = [jax.device_put(a, sh) for a in concat_in]

        def zeros():
            return [jax.device_put(np.zeros((self.n_cores * a.shape[0], *a.shape[1:]), a.dtype), sh)
                    for a in self.out_avals]

        out_arrs = jax.block_until_ready(self.sharded(*concat_in, *zeros()))
        times = []
        for _ in range(n_timed):
            z = zeros()
            jax.block_until_ready(z)
            t0 = time.perf_counter()
            out_arrs = jax.block_until_ready(self.sharded(*concat_in, *z))
            times.append(time.perf_counter() - t0)
        results = [
            {name: np.asarray(out_arrs[i]).reshape(self.n_cores, *self.out_avals[i].shape)[c]
             for i, name in enumerate(self.out_names)}
            for c in range(self.n_cores)
        ]
        return results, times


_RUNNER_CACHE = {}


def get_runner(repeats=1):
    if repeats not in _RUNNER_CACHE:
        _RUNNER_CACHE[repeats] = SpmdRunner(build_kernel(repeats=repeats), N_CORES)
    return _RUNNER_CACHE[repeats]


def kernel(**inputs) -> np.ndarray:
    run = get_runner(repeats=1)
    in_maps = pack_inputs(inputs)
    results, _ = run(in_maps)
    return unpack_outputs(results).astype(np.float32)



# revision 7
# speedup vs baseline: 1.4323x; 1.4323x over previous
"""TRN2 Bass kernel for nn_PosOnlyModel: 2-layer LSTM encoder (15 steps) +
autoregressive 2-layer LSTM decoder (25 steps) with Linear head and
unit-sphere residual position updates.

v2: pure data parallel across 8 NeuronCores (batch 8192 -> 1024/core).
Per core, two half-batch streams of 512 pipeline against each other so one
stream's recurrence handoff latency hides behind the other stream's engine
work. Hidden-state matmuls run as fp8e4m3 DoubleRow PE passes (two 128-K
tiles per instruction); layer-0 K=3 input passes stay fp16 with the bias
folded in as a fourth lhsT row against a ones row in the rhs; layer-1 bias
is a rank-1 fp16 pass. Gate ACT order is [g,i,f,o] so the DVE c-update
chain (fp16, per-hb) finishes before ACT needs tanh(c); h = o*tanh(c) is
written to fp8 on Pool (off the DVE queue). The decoder head runs its
normalize chain on Pool (Taylor+Newton rsqrt around ||pos||~1), and the
emission is software-pipelined so PE never queues behind a not-yet-ready
head while cell work is available.
"""
import sys
import numpy as np

sys.path.insert(0, '/opt/trn_rl_repo')

import concourse.bass as bass
import concourse.bacc as bacc
import concourse.mybir as mybir
from concourse.tile import TileContext

dt = mybir.dt
F32 = dt.float32
F16 = dt.float16
F8 = dt.float8e4
AF = mybir.ActivationFunctionType
ALU = mybir.AluOpType
AX = mybir.AxisListType
DR = mybir.MatmulPerfMode.DoubleRow

H = 256
T_ENC = 15
T_DEC = 25
N_CORES = 8
B_FULL = 8192
B = B_FULL // N_CORES
NS = 2                  # streams per core
BS = B // NS            # 512, one matmul free-dim chunk
NK = 2                  # 128-wide K tiles in H
NB = B // 128           # 8 batch-major tiles per core
NBS = NB // NS          # 4 per stream
MG_ORDER = (2, 0, 1, 3)  # g first, then i, f, o


def build_kernel(repeats=1):
    nc = bacc.Bacc("TRN2", target_bir_lowering=False)

    enc_x = nc.declare_dram_parameter("enc_x", [4, T_ENC * B], F16, isOutput=False)
    pos0 = nc.declare_dram_parameter("pos0", [4, B], F16, isOutput=False)
    pos0bm = nc.declare_dram_parameter("pos0bm", [128, NB * 3], F32, isOutput=False)
    w_hh, w_ih = {}, {}
    for cell in ("e0", "e1", "d0", "d1"):
        w_hh[cell] = nc.declare_dram_parameter(f"whh_{cell}", [128, NK * 8 * 128], F8, isOutput=False)
        if cell in ("e1", "d1"):
            w_ih[cell] = nc.declare_dram_parameter(f"wih_{cell}", [128, NK * 8 * 128], F8, isOutput=False)
        else:
            # rows 0-2: Wih.T ; row 3: fused bias (bih+bhh)
            w_ih[cell] = nc.declare_dram_parameter(f"wih_{cell}", [4, 8 * 128], F16, isOutput=False)
    bias32_d = nc.declare_dram_parameter("bias32", [128, 16], F32, isOutput=False)
    fcw_d = nc.declare_dram_parameter("fcw", [128, NK * 4], F16, isOutput=False)
    fcbbm_d = nc.declare_dram_parameter("fcbbm", [128, NB * 3], F32, isOutput=False)
    ident_d = nc.declare_dram_parameter("ident", [128, 128], F32, isOutput=False)
    ys_d = nc.declare_dram_parameter("ys", [128, T_DEC * NB * 3], F32, isOutput=True)

    with TileContext(nc) as tc:
        with tc.tile_pool(name="wpool", bufs=1) as wp, \
             tc.tile_pool(name="state", bufs=1) as sp, \
             tc.tile_pool(name="acts", bufs=12) as ap, \
             tc.tile_pool(name="tmps", bufs=8) as tp, \
             tc.tile_pool(name="xp", bufs=3) as xp, \
             tc.tile_pool(name="smalls", bufs=2) as mp, \
             tc.tile_pool(name="psum", bufs=3, space="PSUM") as pp, \
             tc.tile_pool(name="psumh", bufs=2, space="PSUM") as hp:

            dmaeng = [nc.sync, nc.gpsimd, nc.scalar]
            di = [0]

            def dma(out, in_):
                dmaeng[di[0] % len(dmaeng)].dma_start(out=out, in_=in_)
                di[0] += 1

            whh_t, wih_t, bias1_t = {}, {}, {}
            for cell in ("e0", "d0"):
                wih_t[cell] = wp.tile([4, 8 * 128], F16, name=f"wih_{cell}")
                dma(wih_t[cell][:], w_ih[cell][:])
            for cell in ("e1", "d1"):
                wih_t[cell] = wp.tile([128, NK, 8 * 128], F8, name=f"wih_{cell}")
                dma(wih_t[cell][:], w_ih[cell][:].rearrange("p (k m) -> p k m", k=NK))
            for cell in ("e0", "e1", "d0", "d1"):
                whh_t[cell] = wp.tile([128, NK, 8 * 128], F8, name=f"whh_{cell}")
                dma(whh_t[cell][:], w_hh[cell][:].rearrange("p (k m) -> p k m", k=NK))
            bias32_t = wp.tile([128, 16], F32, name="bias32")
            dma(bias32_t[:], bias32_d[:])
            fcw_t = wp.tile([128, NK, 4], F16, name="fcw")
            dma(fcw_t[:], fcw_d[:].rearrange("p (k m) -> p k m", k=NK))
            fcbbm_t = wp.tile([128, NB * 3], F32, name="fcbbm")
            dma(fcbbm_t[:], fcbbm_d[:])
            ident_t = wp.tile([128, 128], F32, name="ident")
            dma(ident_t[:], ident_d[:])
            pos0bm_t = wp.tile([128, NB * 3], F32, name="pos0bm")
            dma(pos0bm_t[:], pos0bm[:])

            # state per [layer][stream][parity]
            h_t = [[[sp.tile([128, NK, BS], F8, name=f"h{l}_{st}_{par}")
                     for par in range(2)] for st in range(NS)] for l in range(2)]
            c_t = [[sp.tile([128, NK, BS], F16, name=f"c{l}_{st}") for st in range(NS)]
                   for l in range(2)]
            h1fc_t = [sp.tile([128, NK, BS], F16, name=f"h1fc_{st}") for st in range(NS)]

            def emit_cell(cell, layer, s, st, x_ap, h_rd, h_wr, c_rw, h16_wr=None):
                """One LSTM cell update for one stream.

                h_rd: list of (w_tile_f8, h_tile_f8) DoubleRow passes.
                x_ap: [4, BS] fp16 (rows 0-2 data, row 3 ones) for layer 0.
                Layer 1's bias is a rank-1 pass bias1_t[cell] x ones_t.
                """
                first = s == 0
                ps_t, acts = {}, {}
                # phase 1: recurrent DoubleRow passes (deps: prev-step h, ready early)
                for mg in MG_ORDER:
                    ps = pp.tile([128, 2, BS], F32, name="ps_gates", tag="ps_gates")
                    ps_t[mg] = ps
                    for hb in range(2):
                        m = hb * 4 + mg
                        ms = slice(m * 128, (m + 1) * 128)
                        for pi, (w, rhs) in enumerate(h_rd):
                            nc.tensor.matmul(ps[:, hb, :], w[:, 0:2, ms], rhs[:, 0:2, :],
                                             start=(pi == 0),
                                             stop=(layer == 1 and pi == len(h_rd) - 1),
                                             perf_mode=DR)
                # phase 2: layer 0's input pass closes each group (bias rides as
                # the ones-row); layer 1 closes on the ih pass and fuses bias in
                # per-hb ACT instrs instead (keeps PE work minimal).
                bcol = 8 if cell == "d1" else 0
                for mg in MG_ORDER:
                    ps = ps_t[mg]
                    a = ap.tile([128, 2, BS], F16, name="act", tag="act")
                    func = AF.Tanh if mg == 2 else AF.Sigmoid
                    if layer == 0:
                        for hb in range(2):
                            m = hb * 4 + mg
                            ms = slice(m * 128, (m + 1) * 128)
                            nc.tensor.matmul(ps[:, hb, :], wih_t[cell][:, ms], x_ap,
                                             start=first, stop=True)
                        nc.scalar.activation(a[:], ps[:], func)
                    else:
                        for hb in range(2):
                            m = hb * 4 + mg
                            nc.scalar.activation(a[:, hb, :], ps[:, hb, :], func,
                                                 bias=bias32_t[:, bcol + m: bcol + m + 1])
                        acts[mg] = a
                    acts[mg] = a
                a_i, a_f, a_g, a_o = acts[0], acts[1], acts[2], acts[3]
                # c update on DVE (fp16, per-hb so tanh(c) can start early)
                if first:
                    for hb in range(2):
                        nc.vector.tensor_mul(c_rw[:, hb, :], a_i[:, hb, :], a_g[:, hb, :])
                else:
                    t2 = tp.tile([128, 2, BS], F16, name="t2", tag="tmp")
                    t1 = tp.tile([128, 2, BS], F16, name="t1", tag="tmp")
                    nc.vector.tensor_mul(t2[:, 0, :], a_i[:, 0, :], a_g[:, 0, :])
                    nc.vector.tensor_mul(t2[:, 1, :], a_i[:, 1, :], a_g[:, 1, :])
                    nc.vector.tensor_mul(t1[:, 0, :], a_f[:, 0, :], c_rw[:, 0, :])
                    nc.vector.tensor_add(c_rw[:, 0, :], t1[:, 0, :], t2[:, 0, :])
                    nc.vector.tensor_mul(t1[:, 1, :], a_f[:, 1, :], c_rw[:, 1, :])
                    nc.vector.tensor_add(c_rw[:, 1, :], t1[:, 1, :], t2[:, 1, :])
                t3 = tp.tile([128, 2, BS], F16, name="t3", tag="tmp3")
                nc.scalar.activation(t3[:, 0, :], c_rw[:, 0, :], AF.Tanh)
                nc.scalar.activation(t3[:, 1, :], c_rw[:, 1, :], AF.Tanh)
                # h (fp8, for next matmuls) on DVE; Pool is reserved for the
                # decoder head chains so they never queue behind h-muls.
                if h16_wr is not None:
                    nc.vector.tensor_mul(h16_wr[:], a_o[:], t3[:])
                nc.vector.tensor_mul(h_wr[:, 0, :], a_o[:, 0, :], t3[:, 0, :])
                nc.vector.tensor_mul(h_wr[:, 1, :], a_o[:, 1, :], t3[:, 1, :])

            CELLS = {0: ("e0", "e1"), 1: ("d0", "d1")}

            for rep in range(repeats):
                pos_mm = [None] * NS
                pos_bm = [None] * NS
                psd_t = [None] * NS

                def emit_c0(s, st, x_ap):
                    p, q = s % 2, 1 - s % 2
                    c0n = CELLS[1 if s >= T_ENC else 0][0]
                    h_rd = [] if s == 0 else [(whh_t[c0n], h_t[0][st][q])]
                    emit_cell(c0n, 0, s, st, x_ap, h_rd, h_t[0][st][p], c_t[0][st])

                def emit_c1(s, st):
                    p, q = s % 2, 1 - s % 2
                    is_dec = s >= T_ENC
                    c1n = CELLS[1 if is_dec else 0][1]
                    h_rd = [(wih_t[c1n], h_t[0][st][p])]
                    if s != 0:
                        h_rd = [(whh_t[c1n], h_t[1][st][q])] + h_rd
                    emit_cell(c1n, 1, s, st, None, h_rd, h_t[1][st][p], c_t[1][st],
                              h16_wr=h1fc_t[st] if is_dec else None)

                def emit_head_fc(st):
                    psd = hp.tile([128, NBS, 4], F32, name="ps_fc", tag="ps_head")
                    for m in range(NBS):
                        for k in range(NK):
                            nc.tensor.matmul(psd[:, m, :],
                                             h1fc_t[st][:, k, m * 128:(m + 1) * 128],
                                             fcw_t[:, k, :],
                                             start=(k == 0), stop=(k == NK - 1))
                    psd_t[st] = psd

                def emit_head_chain(st, d):
                    """Normalize chain on Pool; PE transposes; produces pos for step d+1."""
                    last = d == T_DEC - 1
                    m0 = st * NBS
                    c3 = slice(m0 * 3, (m0 + NBS) * 3)
                    if pos_bm[st] is None:
                        pb = mp.tile([128, NBS * 3], F32, name="pfcb", tag=f"pfcb{st}")
                        nc.gpsimd.tensor_add(pb[:], pos0bm_t[:, c3], fcbbm_t[:, c3])
                        pos_bm[st] = pb
                    npos = mp.tile([128, NBS * 3], F32, name="npos", tag=f"npos{st}")
                    nc.vector.tensor_add(npos[:].rearrange("p (m c) -> p m c", c=3),
                                         psd_t[st][:, :, 0:3],
                                         pos_bm[st][:].rearrange("p (m c) -> p m c", c=3))
                    sq = mp.tile([128, NBS * 3], F32, name="sq", tag=f"sm{st}")
                    nc.gpsimd.tensor_mul(sq[:], npos[:], npos[:])
                    sq3 = sq[:].rearrange("p (m c) -> p m c", c=3)
                    ss = mp.tile([128, NBS], F32, name="ss", tag=f"sm8{st}")
                    nc.gpsimd.tensor_add(ss[:], sq3[:, :, 0], sq3[:, :, 1])
                    nc.gpsimd.tensor_add(ss[:], ss[:], sq3[:, :, 2])
                    # rsqrt(ss) with ss ~ 1: y0 = 1.5 - 0.5*ss, one Newton step
                    y0 = mp.tile([128, NBS], F32, name="y0", tag=f"sm8b{st}")
                    nc.gpsimd.tensor_scalar(out=y0[:], in0=ss[:], scalar1=-0.5, scalar2=1.5,
                                            op0=ALU.mult, op1=ALU.add)
                    t = mp.tile([128, NBS], F32, name="nrt", tag=f"sm8c{st}")
                    nc.gpsimd.tensor_mul(t[:], y0[:], y0[:])
                    nc.gpsimd.tensor_mul(t[:], t[:], ss[:])
                    nc.gpsimd.tensor_scalar(out=t[:], in0=t[:], scalar1=-0.5, scalar2=1.5,
                                            op0=ALU.mult, op1=ALU.add)
                    y = mp.tile([128, NBS], F32, name="y", tag=f"sm8d{st}")
                    nc.gpsimd.tensor_mul(y[:], y0[:], t[:])
                    posn = mp.tile([128, NBS * 3], F32, name="posn", tag=f"ysst{st}")
                    yb = y[:].unsqueeze(2).broadcast_to([128, NBS, 3])
                    nc.gpsimd.tensor_tensor(out=posn[:].rearrange("p (m c) -> p m c", c=3),
                                            in0=npos[:].rearrange("p (m c) -> p m c", c=3),
                                            in1=yb, op=ALU.mult)
                    nc.sync.dma_start(out=ys_d[:, d * NB * 3 + m0 * 3: d * NB * 3 + (m0 + NBS) * 3],
                                      in_=posn[:])
                    if not last:
                        pb = mp.tile([128, NBS * 3], F32, name="pfcb", tag=f"pfcb{st}")
                        nc.gpsimd.tensor_add(pb[:], posn[:], fcbbm_t[:, c3])
                        pos_bm[st] = pb
                        ps_tr = hp.tile([3, BS], F32, name="ps_tr", tag="ps_head")
                        for m in range(NBS):
                            nc.tensor.transpose(ps_tr[:, m * 128:(m + 1) * 128],
                                                posn[:, m * 3:(m + 1) * 3], ident_t[:])
                        pm = mp.tile([4, BS], F16, name="posmm", tag=f"smr{st}")
                        nc.gpsimd.memset(pm[:], 1.0)
                        nc.vector.tensor_copy(out=pm[0:3, :], in_=ps_tr[:])
                        pos_mm[st] = pm

                # ---- encoder ----
                for s in range(T_ENC):
                    xt = xp.tile([4, B], F16, name="xst", tag="xst")
                    nc.sync.dma_start(out=xt[:], in_=enc_x[:, s * B:(s + 1) * B])
                    emit_c0(s, 0, xt[:, 0:BS])
                    emit_c0(s, 1, xt[:, BS:B])
                    emit_c1(s, 0)
                    emit_c1(s, 1)

                # ---- decoder, software-pipelined emission ----
                pos0_t = mp.tile([4, B], F16, name="pos0t", tag="pos0t")
                nc.sync.dma_start(out=pos0_t[:], in_=pos0[:])
                emit_c0(T_ENC, 0, pos0_t[:, 0:BS])
                for d in range(T_DEC):
                    s = T_ENC + d
                    emit_c0(s, 1, pos0_t[:, BS:B] if d == 0 else pos_mm[1][:])
                    emit_c1(s, 0)
                    emit_c1(s, 1)
                    emit_head_fc(0)
                    emit_head_chain(0, d)           # -> pos_mm[0] for step s+1
                    if d < T_DEC - 1:
                        emit_c0(s + 1, 0, pos_mm[0][:])
                    emit_head_fc(1)
                    emit_head_chain(1, d)           # -> pos_mm[1] for step s+1

    nc.finalize()
    return nc


def pack_inputs(inputs):
    import ml_dtypes
    F8NP = ml_dtypes.float8_e4m3fn
    perm = np.concatenate([np.arange(g * 256 + hb * 128, g * 256 + hb * 128 + 128)
                           for hb in range(2) for g in range(4)])

    def pack_whh(w, np_dtype):
        wt = w[perm].T.astype(np_dtype)
        K = wt.shape[0]
        return wt.reshape(K // 128, 128, 1024).transpose(1, 0, 2).reshape(128, -1).copy()

    shared = {}
    bias32 = np.zeros((128, 16), np.float32)
    for cell, pre in (("e0", "enc_"), ("e1", "enc_"), ("d0", "dec_"), ("d1", "dec_")):
        idx = cell[1]
        shared[f"whh_{cell}"] = pack_whh(np.asarray(inputs[f"{pre}Whh{idx}"]), F8NP)
        wih = np.asarray(inputs[f"{pre}Wih{idx}"])
        b = (np.asarray(inputs[f"{pre}bih{idx}"]) + np.asarray(inputs[f"{pre}bhh{idx}"]))[perm]
        if wih.shape[1] == H:
            shared[f"wih_{cell}"] = pack_whh(wih, F8NP)
            bias32[:, (8 if cell == "d1" else 0):][:, 0:8] = b.reshape(8, 128).T
        else:
            w4 = np.zeros((4, 8 * 128), np.float16)
            w4[0:3] = wih[perm].T.astype(np.float16)
            w4[3] = b.astype(np.float16)
            shared[f"wih_{cell}"] = w4
    shared["bias32"] = bias32
    fcw = np.zeros((256, 4), np.float16)
    fcw[:, :3] = np.asarray(inputs["fc_W"]).T.astype(np.float16)
    shared["fcw"] = fcw.reshape(2, 128, 4).transpose(1, 0, 2).reshape(128, 8).copy()
    shared["fcbbm"] = np.tile(np.asarray(inputs["fc_b"]).astype(np.float32), (128, NB)).copy()
    shared["ident"] = np.eye(128, dtype=np.float32)

    enc = np.asarray(inputs["encoder_position_inputs"], np.float32)
    dec = np.asarray(inputs["decoder_position_inputs"], np.float32)
    in_maps = []
    for c in range(N_CORES):
        sl = slice(c * B, (c + 1) * B)
        m = dict(shared)
        ex = np.ones((4, T_ENC, B), np.float32)
        ex[0:3] = enc[sl].transpose(2, 1, 0)
        m["enc_x"] = ex.reshape(4, T_ENC * B).astype(np.float16)
        p0 = dec[sl, 0, :]
        p4 = np.ones((4, B), np.float32)
        p4[0:3] = p0.T
        m["pos0"] = p4.astype(np.float16)
        m["pos0bm"] = p0.reshape(NB, 128, 3).transpose(1, 0, 2).reshape(128, NB * 3).astype(np.float32).copy()
        in_maps.append(m)
    return in_maps


def unpack_outputs(results):
    outs = []
    for c in range(N_CORES):
        ys = results[c]["ys"].reshape(128, T_DEC, NB, 3)
        outs.append(ys.transpose(2, 0, 1, 3).reshape(B, T_DEC, 3))
    return np.concatenate(outs, axis=0)


class SpmdRunner:
    """Compile a finalized Bass module once; run it many times."""

    def __init__(self, nc, n_cores):
        import jax
        from jax.sharding import Mesh, PartitionSpec
        from jax.experimental.shard_map import shard_map
        from concourse.bass2jax import _bass_exec_p, install_neuronx_cc_hook, partition_id_tensor
        self.jax = jax
        self.PartitionSpec = PartitionSpec
        install_neuronx_cc_hook()
        self.nc = nc
        self.n_cores = n_cores
        partition_name = nc.partition_id_tensor.name if nc.partition_id_tensor else None
        in_names, out_names, out_avals = [], [], []
        for alloc in nc.m.functions[0].allocations:
            if not isinstance(alloc, mybir.MemoryLocationSet):
                continue
            name = alloc.memorylocations[0].name
            if alloc.kind == "ExternalInput":
                if name != partition_name:
                    in_names.append(name)
            elif alloc.kind == "ExternalOutput":
                out_names.append(name)
                out_avals.append(jax.core.ShapedArray(tuple(alloc.tensor_shape), mybir.dt.np(alloc.dtype)))
        self.in_names, self.out_names, self.out_avals = in_names, out_names, out_avals
        n_params = len(in_names)
        n_outs = len(out_avals)
        all_in_names = list(in_names) + list(out_names)
        if partition_name is not None:
            all_in_names.append(partition_name)

        def _body(*args):
            operands = list(args)
            if partition_name is not None:
                operands.append(partition_id_tensor())
            outs = _bass_exec_p.bind(
                *operands,
                out_avals=tuple(out_avals),
                in_names=tuple(all_in_names),
                out_names=tuple(out_names),
                lowering_input_output_aliases=(),
                sim_require_finite=True,
                sim_require_nnan=True,
                nc=nc,
            )
            return tuple(outs)

        devices = jax.devices()[:n_cores]
        self.mesh = Mesh(np.asarray(devices), ("core",))
        in_specs = (PartitionSpec("core"),) * (n_params + n_outs)
        out_specs = (PartitionSpec("core"),) * n_outs
        donate = tuple(range(n_params, n_params + n_outs))
        self.sharded = jax.jit(
            shard_map(_body, mesh=self.mesh, in_specs=in_specs, out_specs=out_specs, check_rep=False),
            donate_argnums=donate, keep_unused=True,
        )
        self.n_params, self.n_outs = n_params, n_outs

    def __call__(self, in_maps, n_timed=0):
        import time
        jax = self.jax
        from jax.sharding import NamedSharding
        per_core = [[np.asarray(m[name]) for name in self.in_names] for m in in_maps]
        concat_in = [np.concatenate([per_core[c][i] for c in range(self.n_cores)], axis=0)
                     for i in range(self.n_params)]
        sh = NamedSharding(self.mesh, self.PartitionSpec("core"))
        concat_in = [jax.device_put(a, sh) for a in concat_in]

        def zeros():
            return [jax.device_put(np.zeros((self.n_cores * a.shape[0], *a.shape[1:]), a.dtype), sh)
                    for a in self.out_avals]

        out_arrs = jax.block_until_ready(self.sharded(*concat_in, *zeros()))
        times = []
        for _ in range(n_timed):
            z = zeros()
            jax.block_until_ready(z)
            t0 = time.perf_counter()
            out_arrs = jax.block_until_ready(self.sharded(*concat_in, *z))
            times.append(time.perf_counter() - t0)
        results = [
            {name: np.asarray(out_arrs[i]).reshape(self.n_cores, *self.out_avals[i].shape)[c]
             for i, name in enumerate(self.out_names)}
            for c in range(self.n_cores)
        ]
        return results, times


_RUNNER_CACHE = {}


def get_runner(repeats=1):
    if repeats not in _RUNNER_CACHE:
        _RUNNER_CACHE[repeats] = SpmdRunner(build_kernel(repeats=repeats), N_CORES)
    return _RUNNER_CACHE[repeats]


def kernel(**inputs) -> np.ndarray:
    run = get_runner(repeats=1)
    in_maps = pack_inputs(inputs)
    results, _ = run(in_maps)
    return unpack_outputs(results).astype(np.float32)
